# revision 2
# baseline (speedup 1.0000x reference)
"""Trainium2 Bass kernel v2: chunked-SSD (scan-free) Mamba block.

8 cores = 2 batch x 4 channel-quarters (1024 ch each). Per core,
t-major chunks of Q=128: PE mask-matmuls compute the recentered cumsum
(M-1/2J), per-state weighted causal sums (cumW) with h0 handoff rows,
Y accumulation via identity matmuls, and all projections. DVE does the
q-power chain, B/C folds (2x via permuted channel order
c'' = ci*64 + b*16 + a), and products. Scalar does exp/ln chains and
per-state decay exps exp(-(s+1)*Sh).

Software-pipelined: chunk k+1's projections/chains are emitted in
slices between the s-iterations of chunk k's state loop so the PE
stays fed while DVE works through the serial per-state chain.
"""
import os
import sys

import numpy as np

sys.path.insert(0, "/opt/trn_rl_repo")

import ml_dtypes  # noqa: E402

BF = ml_dtypes.bfloat16

B_, L_, DM, DIN, DXB, DS, DC, DTR = 2, 4096, 2048, 4096, 1024, 16, 4, 128
NCORES = 8
NCH = 4
DIN_SH = 1024
Q = 128
NCHUNK = L_ // Q
BW = 256
NBLK = L_ // BW


def _perm():
    """c''[j] -> local model channel; c'' = ci*64 + b*16 + a, h = 4a+b."""
    p = np.zeros(DIN_SH, np.int64)
    for j in range(DIN_SH):
        ci, b, a = j // 64, (j % 64) // 16, j % 16
        p[j] = (4 * a + b) * 16 + ci
    return p


def _build(apply_d):
    from concourse import bass, mybir
    from concourse.tile import TileContext

    F32, BF16 = mybir.dt.float32, mybir.dt.bfloat16
    AF = mybir.ActivationFunctionType
    OP = mybir.AluOpType

    nc = bass.Bass()
    dp = nc.declare_dram_parameter
    hsT = dp("hsT", [DM, L_], BF16, isOutput=False)
    wxs = dp("wxs", [DM, 256], BF16, isOutput=False)
    wzt = dp("wzt", [DM, DIN_SH], BF16, isOutput=False)
    wct = dp("wct", [DM, DIN_SH], BF16, isOutput=False)
    wbt = dp("wbt", [DM, 256], BF16, isOutput=False)
    wdd = dp("wdd", [DM, DTR], BF16, isOutput=False)
    wdt = dp("wdt", [DTR, DIN_SH], BF16, isOutput=False)
    wout = dp("wout", [DIN_SH, DM], BF16, isOutput=False)
    cvd = dp("cvd", [128, 8 * DC * 128], BF16, isOutput=False)
    bd2 = dp("bd2", [1, DIN_SH], BF16, isOutput=False)
    dtile = dp("dtile", [128, DIN_SH], BF16, isOutput=False)
    masks = dp("masks", [128, 3 * 128], BF16, isOutput=False)
    sel16 = dp("sel16", [16, DS * 128], BF16, isOutput=False)
    scl16 = dp("scl16", [16, 1], F32, isOutput=False)
    outp = dp("outp", [L_, DM], BF16, isOutput=True)

    with TileContext(nc) as tc:
        with tc.tile_pool(name="wp", bufs=1) as wp, \
             tc.tile_pool(name="hsp", bufs=1) as hsp, \
             tc.tile_pool(name="xcp", bufs=1) as xcp, \
             tc.tile_pool(name="ucm", bufs=1) as ucm, \
             tc.tile_pool(name="ck", bufs=1) as ck, \
             tc.tile_pool(name="pk2", bufs=2) as pk2, \
             tc.tile_pool(name="wsb", bufs=2) as wsb, \
             tc.tile_pool(name="nsb", bufs=2) as nsb, \
             tc.tile_pool(name="ms1", bufs=1) as ms1, \
             tc.tile_pool(name="psY", bufs=1, space="PSUM") as psY, \
             tc.tile_pool(name="psW", bufs=2, space="PSUM") as psW, \
             tc.tile_pool(name="psA", bufs=3, space="PSUM") as psA, \
             tc.tile_pool(name="psT", bufs=1, space="PSUM") as psT:

            # ---------- resident weights ----------
            w_xs = wp.tile([128, 16, 256], BF16, tag="w_xs")
            w_zt = wp.tile([128, 16, DIN_SH], BF16, tag="w_zt")
            w_ct = wp.tile([128, 16, DIN_SH], BF16, tag="w_ct")
            w_bt = wp.tile([128, 16, 256], BF16, tag="w_bt")
            w_dd = wp.tile([128, 16, DTR], BF16, tag="w_dd")
            w_dt = wp.tile([128, DIN_SH], BF16, tag="w_dt")
            w_out = wp.tile([128, 8, DM], BF16, tag="w_out")
            w_cvd = wp.tile([128, 8 * DC, 128], BF16, tag="w_cvd")
            nc.sync.dma_start(out=w_xs[:], in_=wxs.rearrange("(kt p) c -> p kt c", p=128))
            nc.sync.dma_start(out=w_zt[:], in_=wzt.rearrange("(kt p) c -> p kt c", p=128))
            nc.sync.dma_start(out=w_ct[:], in_=wct.rearrange("(kt p) c -> p kt c", p=128))
            nc.sync.dma_start(out=w_bt[:], in_=wbt.rearrange("(kt p) c -> p kt c", p=128))
            nc.sync.dma_start(out=w_dd[:], in_=wdd.rearrange("(kt p) r -> p kt r", p=128))
            nc.sync.dma_start(out=w_dt[:], in_=wdt[:])
            nc.sync.dma_start(out=w_out[:], in_=wout.rearrange("(i p) m -> p i m", p=128))
            nc.sync.dma_start(out=w_cvd[:], in_=cvd.rearrange("p (i c) -> p i c", c=128))
            bdt2 = wp.tile([1, DIN_SH], BF16, tag="bdt2")
            nc.sync.dma_start(out=bdt2[:], in_=bd2[:])
            msk = wp.tile([128, 3, 128], BF16, tag="msk")
            nc.sync.dma_start(out=msk[:], in_=masks.rearrange("p (i c) -> p i c", c=128))
            s16 = wp.tile([16, 1], F32, tag="s16")
            nc.sync.dma_start(out=s16[:], in_=scl16[:])
            selt = wp.tile([16, DS, 128], BF16, tag="selt")
            nc.sync.dma_start(out=selt[:], in_=sel16.rearrange(
                "p (s c) -> p s c", c=128))
            dti = None
            if apply_d:
                dti = wp.tile([128, DIN_SH], BF16, tag="dti")
                nc.sync.dma_start(out=dti[:], in_=dtile[:])
            ones1 = wp.tile([1, 128], BF16, tag="ones1")
            nc.vector.memset(ones1[:], 1.0)
            MASK = msk[:, 0, :]
            MASKC = msk[:, 1, :]
            IDY = msk[:, 2, :]

            hsT_r = hsT.rearrange("(kt p) t -> p kt t", p=128)

            blkstate = {}

            def block_slices(blk):
                """Yield emission slices for per-block work (hs, xs, xc,
                conv, u_cm, dlT)."""
                t0 = blk * BW
                hsb = hsp.tile([128, 16, BW], BF16, tag="hsb")
                nc.sync.dma_start(out=hsb[:], in_=hsT_r[:, :, t0:t0 + BW])
                blkstate["hsb"] = hsb
                yield
                xs = xcp.tile([128, 2, BW + 4], BF16, tag="xs")
                for xt in range(2):
                    ps_x = psA.tile([128, 512], F32, tag="psA")
                    for kt in range(16):
                        nc.tensor.matmul(ps_x[:, 0:BW],
                                         lhsT=w_xs[:, kt, xt * 128:(xt + 1) * 128],
                                         rhs=hsb[:, kt, :],
                                         start=(kt == 0), stop=(kt == 15))
                    if blk == 0:
                        nc.vector.memset(xs[:, xt, 0:4], 0.0)
                    else:
                        nc.vector.tensor_copy(xs[:, xt, 0:4], xs[:, xt, BW:BW + 4])
                    nc.vector.tensor_copy(xs[:, xt, 4:BW + 4], ps_x[:, 0:BW])
                    yield
                xc = xcp.tile([128, 8, BW + 4], BF16, tag="xc")
                xsa, xca = xs[:], xc[:]
                spp, dpp = xsa.ap[0][0], xca.ap[0][0]
                for i in range(8):
                    for o in range(2):
                        sr = 32 * i + 16 * o
                        src = bass.AP(
                            tensor=xsa.tensor,
                            offset=(xsa.offset + (sr % 128) * spp
                                    + (sr // 128) * (BW + 4)),
                            ap=[[1 * spp, 16], [1, BW + 4]])
                        for b in range(4):
                            dst = bass.AP(
                                tensor=xca.tensor,
                                offset=(xca.offset + i * (BW + 4)
                                        + (64 * o + 16 * b) * dpp),
                                ap=[[1 * dpp, 16], [1, BW + 4]])
                            nc.sync.dma_start(out=dst, in_=src)
                yield
                u_cm = ucm.tile([128, 8, BW], BF16, tag="u_cm")
                blkstate["u_cm"] = u_cm
                for i in range(8):
                    ps_cv = psA.tile([128, 512], F32, tag="psA")
                    for j in range(DC):
                        nc.tensor.matmul(ps_cv[:, 0:BW],
                                         lhsT=w_cvd[:, i * DC + j, :],
                                         rhs=xc[:, i, 1 + j:BW + 1 + j],
                                         start=(j == 0), stop=(j == DC - 1))
                    e1u = ck.tile([128, 512], BF16, tag="tmpA")
                    nc.scalar.activation(e1u[:, 0:BW], ps_cv[:, 0:BW], AF.Exp,
                                         bias=0.0, scale=-1.0)
                    spu = ck.tile([128, 512], BF16, tag="tmpB")
                    nc.scalar.activation(spu[:, 0:BW], e1u[:, 0:BW], AF.Ln,
                                         bias=1.0, scale=1.0)
                    sgu = ck.tile([128, 512], BF16, tag="tmpA")
                    nc.scalar.activation(sgu[:, 0:BW], spu[:, 0:BW], AF.Exp,
                                         bias=0.0, scale=-1.0)
                    nc.vector.tensor_tensor(out=u_cm[:, i, :], in0=ps_cv[:, 0:BW],
                                            in1=sgu[:, 0:BW], op=OP.mult)
                    if i % 3 == 2:
                        yield
                ps_dl = psA.tile([128, 512], F32, tag="psA")
                for kt in range(16):
                    nc.tensor.matmul(ps_dl[:, 0:BW], lhsT=w_dd[:, kt, :],
                                     rhs=hsb[:, kt, :],
                                     start=(kt == 0), stop=(kt == 15))
                dlT = ucm.tile([128, BW], BF16, tag="dlT")
                nc.vector.tensor_copy(dlT[:], ps_dl[:, 0:BW])
                blkstate["dlT"] = dlT
                yield

            def prologue_slices(k):
                """Yield emission slices computing chunk k's inputs.
                Returns (via closure dict) the tile set for body(k)."""
                kc = k % (BW // 128)
                co = kc * 128
                if kc == 0:
                    for _ in block_slices(k // (BW // 128)):
                        yield
                hsb = blkstate["hsb"]
                u_cm = blkstate["u_cm"]
                dlT = blkstate["dlT"]
                st = {}
                blkstate[("st", k)] = st

                # uT transpose
                ps_t = psT.tile([128, DIN_SH], BF16, tag="psT")
                for i in range(8):
                    nc.tensor.transpose(out=ps_t[:, i * 128:(i + 1) * 128],
                                        in_=u_cm[:, i, co:co + 128],
                                        identity=IDY)
                uT = pk2.tile([128, DIN_SH], BF16, tag="uT")
                nc.vector.tensor_copy(uT[:], ps_t[:])
                st["uT"] = uT
                yield

                # delta chain
                dlt = ck.tile([128, DIN_SH], BF16, tag="dlt")
                for hh in range(2):
                    ps_dr = psA.tile([128, 512], F32, tag="psA")
                    nc.tensor.matmul(ps_dr[:], lhsT=dlT[:, co:co + 128],
                                     rhs=w_dt[:, hh * 512:(hh + 1) * 512],
                                     start=True, stop=False, skip_group_check=True)
                    nc.tensor.matmul(ps_dr[:], lhsT=ones1[:],
                                     rhs=bdt2[0:1, hh * 512:(hh + 1) * 512],
                                     start=False, stop=True, skip_group_check=True)
                    e1p = ck.tile([128, 512], BF16, tag="tmpA")
                    nc.scalar.activation(e1p[:], ps_dr[:], AF.Exp, bias=0.0,
                                         scale=1.0)
                    nc.scalar.activation(dlt[:, hh * 512:(hh + 1) * 512],
                                         e1p[:], AF.Ln, bias=1.0, scale=1.0)
                yield

                # Sh + q + du + ab/ex
                sh = pk2.tile([128, DIN_SH], BF16, tag="sh")
                for hh in range(2):
                    ps_sh = psA.tile([128, 512], F32, tag="psA")
                    nc.tensor.matmul(ps_sh[:], lhsT=MASKC,
                                     rhs=dlt[:, hh * 512:(hh + 1) * 512],
                                     start=True, stop=True)
                    nc.scalar.copy(out=sh[:, hh * 512:(hh + 1) * 512], in_=ps_sh[:])
                st["sh"] = sh
                q = pk2.tile([128, DIN_SH], BF16, tag="q")
                nc.scalar.activation(q[:], sh[:], AF.Exp, bias=0.0, scale=1.0)
                st["q"] = q
                du = ck.tile([128, DIN_SH], BF16, tag="du")
                nc.vector.tensor_tensor(out=du[:], in0=dlt[:], in1=uT[:],
                                        op=OP.mult)
                st["du"] = du
                if k > 0:
                    ab = ck.tile([16, DIN_SH], BF16, tag="ab")
                    aap = sh[:]
                    nc.sync.dma_start(
                        out=ab[:],
                        in_=bass.AP(tensor=aap.tensor,
                                    offset=aap.offset + 127 * aap.ap[0][0],
                                    ap=[[aap.ap[0][0], 1], [0, 16],
                                        [1, DIN_SH]]))
                    ex = pk2.tile([16, DIN_SH], BF16, tag="ex")
                    nc.scalar.activation(ex[:], ab[:], AF.Exp, bias=0.0,
                                         scale=s16[:])
                    st["ex"] = ex
                yield

                # B projection
                bpr = pk2.tile([128, 256], BF16, tag="bpr")
                ps_b = psA.tile([128, 512], F32, tag="psA")
                for kt in range(16):
                    nc.tensor.matmul(ps_b[:, 0:256], lhsT=hsb[:, kt, co:co + 128],
                                     rhs=w_bt[:, kt, :],
                                     start=(kt == 0), stop=(kt == 15))
                nc.scalar.copy(out=bpr[:], in_=ps_b[:, 0:256])
                st["bpr"] = bpr
                yield

                # C projection (2 halves)
                cpr = pk2.tile([128, DIN_SH], BF16, tag="cpr")
                st["cpr"] = cpr
                for hh in range(2):
                    ps_c = psA.tile([128, 512], F32, tag="psA")
                    for kt in range(16):
                        nc.tensor.matmul(ps_c[:], lhsT=hsb[:, kt, co:co + 128],
                                         rhs=w_ct[:, kt, hh * 512:(hh + 1) * 512],
                                         start=(kt == 0), stop=(kt == 15))
                    nc.scalar.copy(out=cpr[:, hh * 512:(hh + 1) * 512], in_=ps_c[:])
                    yield

                # z projection + silu (2 halves)
                sz = pk2.tile([128, DIN_SH], BF16, tag="sz")
                st["sz"] = sz
                for hh in range(2):
                    ps_z = psA.tile([128, 512], F32, tag="psA")
                    for kt in range(16):
                        nc.tensor.matmul(ps_z[:], lhsT=hsb[:, kt, co:co + 128],
                                         rhs=w_zt[:, kt, hh * 512:(hh + 1) * 512],
                                         start=(kt == 0), stop=(kt == 15))
                    e1z = ck.tile([128, 512], BF16, tag="tmpA")
                    nc.scalar.activation(e1z[:], ps_z[:], AF.Exp, bias=0.0,
                                         scale=-1.0)
                    spz = ck.tile([128, 512], BF16, tag="tmpB")
                    nc.scalar.activation(spz[:], e1z[:], AF.Ln, bias=1.0, scale=1.0)
                    sgz = ck.tile([128, 512], BF16, tag="tmpA")
                    nc.scalar.activation(sgz[:], spz[:], AF.Exp, bias=0.0,
                                         scale=-1.0)
                    nc.vector.tensor_tensor(
                        out=sz[:, hh * 512:(hh + 1) * 512], in0=ps_z[:],
                        in1=sgz[:], op=OP.mult)
                    yield

            def drain(gen):
                if gen is not None:
                    for _ in gen:
                        pass

            last_prev = None

            # prime chunk 0
            g = prologue_slices(0)
            drain(g)
            nextgen = prologue_slices(1)

            for k in range(NCHUNK):
                st = blkstate.pop(("st", k))
                sh, q, du = st["sh"], st["q"], st["du"]
                bpr, cpr, sz, uT = st["bpr"], st["cpr"], st["sz"], st["uT"]

                # bridge: R0 = last_prev * ex
                r0_cur = None
                if k > 0:
                    r0_cur = ck.tile([16, DIN_SH], BF16, tag="r0t")
                    nc.vector.tensor_tensor(out=r0_cur[:], in0=last_prev[:],
                                            in1=st["ex"][:], op=OP.mult)
                last_cur = ck.tile([16, DIN_SH], BF16, tag="last")

                # ---- per-state loop, interleaved with next chunk's prologue
                T = ck.tile([128, DIN_SH], BF16, tag="Tc")
                psy = psY.tile([128, DIN_SH], F32, tag="psY")
                bap, cap = bpr[:], cpr[:]
                for s in range(DS):
                    nc.vector.tensor_tensor(out=T[:], in0=(du if s == 0 else T)[:],
                                            in1=q[:], op=OP.mult)
                    w_s = wsb.tile([128, DIN_SH], BF16, tag="w_s")
                    b_bc = bass.AP(tensor=bap.tensor, offset=bap.offset + s * 16,
                                   ap=[[bap.ap[0][0], 128], [0, 16], [0, 4],
                                       [1, 16]])
                    nc.vector.tensor_tensor(out=w_s[:], in0=T[:], in1=b_bc,
                                            op=OP.mult)
                    vp = nsb.tile([128, DIN_SH], BF16, tag="vp")
                    nc.scalar.activation(vp[:], sh[:], AF.Exp, bias=0.0,
                                         scale=-float(s + 1))
                    n_s = nsb.tile([128, DIN_SH], BF16, tag="n_s")
                    for hh in range(2):
                        ps_w = psW.tile([128, 512], F32, tag="psW")
                        nc.tensor.matmul(ps_w[:], lhsT=MASK,
                                         rhs=w_s[:, hh * 512:(hh + 1) * 512],
                                         start=True, stop=(k == 0),
                                         skip_group_check=True)
                        if k > 0:
                            nc.tensor.matmul(ps_w[:], lhsT=selt[:, s, :],
                                             rhs=r0_cur[:, hh * 512:(hh + 1) * 512],
                                             start=False, stop=True,
                                             skip_group_check=True)
                        m_s = nsb.tile([128, 512], BF16, tag="m_s")
                        nc.vector.tensor_tensor(
                            out=m_s[:], in0=vp[:, hh * 512:(hh + 1) * 512],
                            in1=ps_w[:], op=OP.mult)
                        msa = m_s[:]
                        nc.sync.dma_start(
                            out=last_cur[s:s + 1, hh * 512:(hh + 1) * 512],
                            in_=bass.AP(tensor=msa.tensor,
                                        offset=msa.offset + 127 * msa.ap[0][0],
                                        ap=[[msa.ap[0][0], 1], [1, 512]]))
                        c_bc = bass.AP(tensor=cap.tensor,
                                       offset=cap.offset + s * 64,
                                       ap=[[cap.ap[0][0], 128], [0, 8],
                                           [16, 4], [1, 16]])
                        nc.vector.tensor_tensor(
                            out=n_s[:, hh * 512:(hh + 1) * 512], in0=m_s[:],
                            in1=c_bc, op=OP.mult)
                    for hh in range(2):
                        nc.tensor.matmul(psy[:, hh * 512:(hh + 1) * 512],
                                         lhsT=IDY,
                                         rhs=n_s[:, hh * 512:(hh + 1) * 512],
                                         start=(s == 0), stop=(s == DS - 1),
                                         skip_group_check=True)
                    # interleave next chunk's prologue
                    if nextgen is not None:
                        next(nextgen, None)

                drain(nextgen)
                nextgen = prologue_slices(k + 2) if k + 2 < NCHUNK else None

                # ---- gating
                yq = ck.tile([128, DIN_SH], BF16, tag="gat")
                if apply_d:
                    utd = ck.tile([128, DIN_SH], BF16, tag="utd")
                    nc.vector.tensor_tensor(out=utd[:], in0=uT[:], in1=dti[:],
                                            op=OP.mult)
                    nc.vector.tensor_tensor(out=yq[:], in0=psy[:], in1=utd[:],
                                            op=OP.add)
                else:
                    nc.vector.tensor_tensor(out=yq[:], in0=psy[:], in1=uT[:],
                                            op=OP.add)
                nc.vector.tensor_tensor(out=yq[:], in0=yq[:], in1=sz[:],
                                        op=OP.mult)

                ps_yt = psT.tile([128, DIN_SH], BF16, tag="psT")
                for i in range(8):
                    nc.tensor.transpose(out=ps_yt[:, i * 128:(i + 1) * 128],
                                        in_=yq[:, i * 128:(i + 1) * 128],
                                        identity=IDY)
                yfc = ck.tile([128, 8, 128], BF16, tag="yfc2")
                nc.vector.tensor_copy(yfc[:], ps_yt[:])

                for dmb in range(4):
                    ps_o = psA.tile([128, 512], F32, tag="psA")
                    for i in range(8):
                        nc.tensor.matmul(ps_o[:], lhsT=yfc[:, i, :],
                                         rhs=w_out[:, i, dmb * 512:(dmb + 1) * 512],
                                         start=(i == 0), stop=(i == 7))
                    o_sb = nsb.tile([128, 512], BF16, tag="m_s")
                    if dmb % 2 == 0:
                        nc.vector.tensor_copy(o_sb[:], ps_o[:])
                    else:
                        nc.scalar.copy(out=o_sb[:], in_=ps_o[:])
                    nc.sync.dma_start(
                        out=outp[k * 128:(k + 1) * 128, dmb * 512:(dmb + 1) * 512],
                        in_=o_sb[:])

                last_prev = last_cur
    return nc


def _prep_inputs(inputs):
    hs = np.asarray(inputs["hidden_states"], np.float32)
    Wx = np.asarray(inputs["Wx"], np.float32)
    Wz = np.asarray(inputs["Wz"], np.float32)
    conv_w = np.asarray(inputs["conv_w"], np.float32)
    WB = np.asarray(inputs["WB"], np.float32)
    WC = np.asarray(inputs["WC"], np.float32)
    Wdd = np.asarray(inputs["Wdt_down"], np.float32)
    Wdt = np.asarray(inputs["Wdt"], np.float32)
    bdt = np.asarray(inputs["bdt"], np.float32)
    A = -np.exp(np.asarray(inputs["A_log"], np.float32))
    D = np.asarray(inputs["D"], np.float32)
    Wout = np.asarray(inputs["Wout"], np.float32)

    assert np.allclose(A, -np.tile(np.arange(1, DS + 1, dtype=np.float32),
                                   (DIN, 1)), atol=1e-4), "A structure"
    apply_d = not np.allclose(D, 1.0)

    p = _perm()
    M = np.triu(np.ones((128, 128), np.float32))
    masks = np.zeros((128, 3, 128), np.float32)
    masks[:, 0, :] = M
    masks[:, 1, :] = M - 0.5
    masks[:, 2, :] = np.eye(128)
    scl16 = -(np.arange(1, DS + 1, dtype=np.float32))[:, None]
    sel = np.zeros((16, DS, 128), np.float32)
    for s in range(DS):
        sel[s, s, :] = 1.0
    r_ = np.arange(256)
    perm16 = (r_ % 16) * 16 + r_ // 16

    WCr = WC.reshape(-1, DS, DM)
    WBr = WB.reshape(-1, DS, DM)

    in_maps = []
    for core in range(NCORES):
        bi, ciq = core // NCH, core % NCH
        ch0 = ciq * DIN_SH
        cg = ch0 + p
        h_lo = ch0 // DS
        bh0 = h_lo // 4

        wct_cols = np.zeros((DIN_SH, DM), np.float32)
        for s in range(DS):
            for b in range(4):
                for a in range(16):
                    wct_cols[s * 64 + b * 16 + a] = WCr[h_lo + 4 * a + b, s]
        wbt_cols = np.zeros((256, DM), np.float32)
        for s in range(DS):
            for a in range(16):
                wbt_cols[s * 16 + a] = WBr[bh0 + a, s]

        cvdt = np.zeros((128, 8 * DC, 128), BF)
        w4 = conv_w[cg, 0, :]
        for i in range(8):
            for j in range(DC):
                cvdt[np.arange(128), i * DC + j, np.arange(128)] = \
                    w4[i * 128:(i + 1) * 128, j].astype(BF)

        in_maps.append({
            "hsT": np.ascontiguousarray(hs[bi].T).astype(BF),
            "wxs": np.ascontiguousarray(Wx[ciq * 256 + perm16].T).astype(BF),
            "wzt": np.ascontiguousarray(Wz[cg].T).astype(BF),
            "wct": np.ascontiguousarray(wct_cols.T).astype(BF),
            "wbt": np.ascontiguousarray(wbt_cols.T).astype(BF),
            "wdd": np.ascontiguousarray(Wdd.T).astype(BF),
            "wdt": np.ascontiguousarray(Wdt[cg].T).astype(BF),
            "wout": np.ascontiguousarray(Wout[:, cg].T).astype(BF),
            "cvd": cvdt.reshape(128, 8 * DC * 128),
            "bd2": (2.0 * bdt[cg])[None, :].astype(BF),
            "dtile": np.tile(D[cg][None, :], (128, 1)).astype(BF),
            "masks": masks.reshape(128, 3 * 128).astype(BF),
            "sel16": sel.reshape(16, DS * 128).astype(BF),
            "scl16": scl16,
        })
    return in_maps, apply_d


def _legalize_waits(nc):
    from concourse import mybir
    n = 0
    for fn in nc.m.functions:
        for blk in fn.blocks:
            newi = []
            for ins in blk.instructions:
                si = ins.sync_info
                if si is not None and si.on_wait is not None and len(si.on_wait) > 1:
                    for w in si.on_wait[:-1]:
                        ev = mybir.InstEventSemaphore(
                            name=f"W-{n}", ins=[], outs=[],
                            sync_info=mybir.SyncInfo(on_wait=[w], on_update=[]))
                        ev.engine = ins.engine
                        newi.append(ev)
                        n += 1
                    si.on_wait = [si.on_wait[-1]]
                newi.append(ins)
            blk.instructions = newi
    return n


def _install_profile_hook():
    import contextlib
    import ctypes
    import types

    import concourse.bass_utils as bu
    bu.upload_artifacts = lambda d: d

    if "antenv.axon_hooks" not in sys.modules:
        mod = types.ModuleType("antenv.axon_hooks")
        _store = {}
        mod.set_axon_ntff_profile_hook = lambda h: _store.__setitem__("h", h)
        mod.get_axon_ntff_profile_hook = lambda: _store.get("h")
        sys.modules["antenv.axon_hooks"] = mod
        import antenv
        antenv.axon_hooks = mod

    from antenv.axon_hooks import (get_axon_ntff_profile_hook,
                                   set_axon_ntff_profile_hook)
    if get_axon_ntff_profile_hook() is not None:
        return
    lib = ctypes.CDLL("/opt/axon/libaxon_pjrt.so")
    if not hasattr(lib, "axon_start_nrt_profile"):
        return
    lib.axon_start_nrt_profile.argtypes = [ctypes.POINTER(ctypes.c_int64),
                                           ctypes.c_size_t]
    lib.axon_start_nrt_profile.restype = ctypes.c_int64
    lib.axon_stop_nrt_profile.argtypes = [ctypes.c_char_p]
    lib.axon_stop_nrt_profile.restype = ctypes.c_int64

    @contextlib.contextmanager
    def _hook(output_dir, device_ids):
        import jax
        jax.devices()
        if device_ids:
            ids = (ctypes.c_int64 * len(device_ids))(*device_ids)
            rc = lib.axon_start_nrt_profile(ids, len(device_ids))
        else:
            rc = lib.axon_start_nrt_profile(None, 0)
        if rc != 0:
            raise RuntimeError(f"axon_start_nrt_profile rc={rc}")
        try:
            yield
        finally:
            n = lib.axon_stop_nrt_profile(str(output_dir).encode())
            print(f"profile: {n} file(s) written to {output_dir}")

    set_axon_ntff_profile_hook(_hook)


def kernel(**inputs):
    from concourse.bass_utils import run_bass_kernel_spmd

    in_maps, apply_d = _prep_inputs(inputs)
    nc = _build(apply_d)
    _legalize_waits(nc)
    trace = bool(int(os.environ.get("MAMBA_PROFILE", "0")))
    tmpdir = None
    if trace:
        import tempfile
        _install_profile_hook()
        tmpdir = tempfile.mkdtemp(prefix="mamba2_trace_")
        kernel.last_trace_dir = tmpdir
    res = run_bass_kernel_spmd(nc, in_maps, core_ids=list(range(NCORES)),
                               trace=trace, tmpdir=tmpdir)
    if trace:
        kernel.last_exec_time_ns = res.exec_time_ns
        kernel.last_profile = res
    bout = np.asarray(inputs["bout"], np.float32)
    out = np.zeros((B_, L_, DM), np.float32)
    for bi in range(B_):
        acc = np.zeros((L_, DM), np.float32)
        for ci in range(NCH):
            acc += np.asarray(res.results[bi * NCH + ci]["outp"], np.float32)
        out[bi] = acc + bout[None, :]
    return out
